# revision 3
# baseline (speedup 1.0000x reference)
"""Trainium2 Bass kernel for NeuralComplexityLoss (sample-entropy MSE).

Contract: kernel(predictions, targets) -> np.float32 scalar (shape ()),
matching reference.reference(). Self-contained: hardcodes shapes/sharding.

Structure (128 signals sharded 16 per core across 8 NeuronCores; per core,
per signal, upper-triangle match counts over strips of 126 template rows):

  ACT  : a = |x_j - x_i|  — ONE abs pass per strip (the baseline computed the
         distance matrix three times, once per template offset k; the three
         are shifted copies of one matrix).  scale=-1 folds the negation;
         per-partition bias columns x_i are gathered once per core.
  DVE  : im = (a <= R) * triangle_mask  -> fp16 (exact 0/1)
  PE   : ps = [4*I0] + 2*I1' + 1*I2''  — weighted shift-matrix matmuls
         accumulated in one PSUM group.  Compute engines cannot read operands
         at partition offsets (32-alignment rule), but the PE's stationary
         operand can encode the partition shift; column shifts are free-dim
         slices of im.  'a' strips carry the 4*I0 term (3 matmuls), 'd'
         strips skip it (2 matmuls) and fold I0 into the DVE predicate.
  preds: cnt2/cnt3 row sums with fused accumulate —
         'a' strips on ACT: Relu(ps-5) sums cnt2+cnt3, Relu(ps-6) sums cnt3;
         'd' strips on DVE: (ps' >= 2)*im and (ps' >= 3)*im via s_t_t.

Host reduces the per-strip row counts, applies the 2c-N symmetry, entropies,
and the final MSE.  A For_i hardware loop (_reps) repeats the whole signal
loop for wall-clock-differencing timing: per-rep slope = HW exec time.
"""

import numpy as np

B, C, T = 4, 16, 1024
M = 2
R = 0.2
EPS = 1e-8
N = T - M                      # 1022 templates
NCORES = 8
NSIG = 2 * B * C               # 128 signals total
S_PER_CORE = NSIG // NCORES    # 16
STRIP = 126
NSTRIPS = 9
PS_CHUNK = 512                 # matmul fp32 PSUM output: one bank = 512 cols
STATS_COLS = S_PER_CORE * NSTRIPS * 2  # 288

# per-strip pipeline type: 'a' = ACT predicates (3-term ps), 'd' = DVE
# predicates (2-term ps, I0 via the s_t_t in1 slot).  GpSimd is useless here:
# walrus rejects scalar_tensor_tensor on Pool and its tensor_scalar ucode is
# an order of magnitude off the cost model.
STRIP_TYPE = ("a", "a", "a", "d", "d", "d", "d", "d", "a")

_CACHE = {}
LAST_RESULTS = None


def _split_excess_waits(nc, maxw=1):
    """Walrus codegen accepts only one sync-wait per instruction: hoist
    extras onto preceding single-wait NOPs on the same engine."""
    import bass_rust
    import concourse.mybir as mybir

    n_split = 0
    for bb in nc.main_func.blocks:
        insts = bb.instructions
        i = 0
        while i < len(insts):
            ins = insts[i]
            si = ins.sync_info
            waits = list(si.on_wait) if si is not None and si.on_wait else []
            if len(waits) > maxw:
                extra, keep = waits[:-maxw], waits[-maxw:]
                nops = []
                for j, w in enumerate(extra):
                    nop = bass_rust.InstNoOp(
                        name=f"{ins.name}-wsplit{j}", ins=[], outs=[]
                    )
                    nop.engine = ins.engine
                    nop.sync_info = mybir.SyncInfo(on_wait=[w], on_update=[])
                    nops.append(nop)
                si.on_wait = keep
                insts[i:i] = nops
                i += len(nops)
                n_split += 1
            i += 1
    return n_split


def _build(reps=1, stype=STRIP_TYPE, bufs=4, psum_bufs=4):
    import concourse.bass as bass
    import concourse.tile as tile
    from concourse import mybir
    from concourse.alu_op_type import AluOpType

    f32 = mybir.dt.float32
    f16 = mybir.dt.float16
    nc = bass.Bass(trn_type="TRN2", num_devices=NCORES)
    x = nc.dram_tensor("x", [S_PER_CORE, T], f32, kind="ExternalInput")
    out = nc.dram_tensor("cnt", [128, STATS_COLS], f32, kind="ExternalOutput")

    mask_np = np.triu(np.ones((128, 1024), dtype=np.float32))
    mask_dram = nc.inline_tensor(mask_np, name="trimask")
    s_np = np.zeros((3, 128, 128), dtype=np.float16)
    for k, w in enumerate((4.0, 2.0, 1.0)):   # ps = 4*I0 + 2*I1 + 1*I2
        for p in range(128 - k):
            s_np[k, p + k, p] = w
    s_dram = [nc.inline_tensor(s_np[k], name=f"shiftw{k}") for k in range(3)]

    xa = x.ap()
    with tile.TileContext(nc) as tc:
        with (
            tc.tile_pool(name="singles", bufs=1) as singles,
            tc.tile_pool(name="xrep", bufs=bufs) as xrep,
            tc.tile_pool(name="apool", bufs=bufs) as apool,
            tc.tile_pool(name="impool", bufs=bufs) as impool,
            tc.tile_pool(name="scr", bufs=2 * bufs) as scr,
            tc.tile_pool(name="ps", bufs=psum_bufs, space="PSUM") as pspool,
        ):
            mask = singles.tile([128, 1024], f32)
            nc.sync.dma_start(out=mask, in_=mask_dram[:, :])
            stw = singles.tile([128, 384], f16)
            for k in range(3):
                nc.sync.dma_start(
                    out=stw[:, 128 * k : 128 * k + 128], in_=s_dram[k][:, :]
                )
            # xcol8[p, 8*s + rt] = x[s, 126*rt + p]  (bias columns, strips 0-7)
            xcol8 = singles.tile([128, 128], f32)
            for s in range(S_PER_CORE):
                nc.sync.dma_start(
                    out=xcol8[:, 8 * s : 8 * s + 8],
                    in_=bass.AP(tensor=x, offset=1024 * s, ap=[[1, 128], [126, 8]]),
                )
            # xcolL[p, s] = x[s, 1008 + p]  (bias column, strip 8; 16 rows)
            xcolL = singles.tile([16, 16], f32)
            nc.sync.dma_start(
                out=xcolL,
                in_=bass.AP(tensor=x, offset=1008, ap=[[1, 16], [1024, 16]]),
            )
            stats = singles.tile([128, STATS_COLS], f32)
            nc.vector.memset(stats, 0.0)
            bneg5 = singles.tile([128, 1], f32)
            nc.vector.memset(bneg5, -5.0)
            bneg6 = singles.tile([128, 1], f32)
            nc.vector.memset(bneg6, -6.0)

            def body():
                pending = []
                for s in range(S_PER_CORE):
                    row = xa[s : s + 1, :]
                    x_rep = xrep.tile([128, T], f32)
                    nc.sync.dma_start(
                        out=x_rep,
                        in_=bass.AP(
                            tensor=row.tensor, offset=row.offset, ap=[[0, 128], [1, T]]
                        ),
                    )
                    for rt in range(NSTRIPS):
                        r0 = STRIP * rt
                        nrows = min(STRIP, N - r0)
                        pmax = min(nrows + 2, 128)
                        wc = N - r0
                        wcp2 = wc + 2
                        bias = (
                            xcol8[0:pmax, 8 * s + rt : 8 * s + rt + 1]
                            if rt < 8
                            else xcolL[0:pmax, s : s + 1]
                        )
                        a = apool.tile([128, 1024], f32)
                        nc.scalar.activation(
                            out=a[0:pmax, 0:wcp2],
                            in_=x_rep[0:pmax, r0 : r0 + wcp2],
                            func=mybir.ActivationFunctionType.Abs,
                            bias=bias,
                            scale=-1.0,
                        )
                        ty = stype[rt]
                        im = impool.tile([128, 1024], f16)
                        nc.vector.scalar_tensor_tensor(
                            out=im[0:pmax, 0:wcp2],
                            in0=a[0:pmax, 0:wcp2],
                            scalar=R,
                            in1=mask[0:pmax, 0:wcp2],
                            op0=AluOpType.is_le,
                            op1=AluOpType.mult,
                        )
                        ks = (0, 1, 2) if ty == "a" else (1, 2)
                        ps = pspool.tile([128, 1024], f32)
                        for c0 in range(0, wc, PS_CHUNK):
                            cw = min(PS_CHUNK, wc - c0)
                            for k in ks:
                                nc.tensor.matmul(
                                    ps[0:nrows, c0 : c0 + cw],
                                    stw[0:pmax, 128 * k : 128 * k + nrows],
                                    im[0:pmax, c0 + k : c0 + k + cw],
                                    start=(k == ks[0]),
                                    stop=(k == ks[-1]),
                                )
                        col2 = (s * NSTRIPS + rt) * 2
                        col3 = col2 + 1

                        def emit_preds(ty=ty, ps=ps, im=im, nrows=nrows, wc=wc,
                                       col2=col2, col3=col3):
                            for which, col in (("c2", col2), ("c3", col3)):
                                o = scr.tile([128, 1024], f16)
                                if ty == "a":
                                    # Relu(ps-5) sums cnt2+cnt3; Relu(ps-6)=cnt3
                                    nc.scalar.activation(
                                        out=o[0:nrows, 0:wc],
                                        in_=ps[0:nrows, 0:wc],
                                        func=mybir.ActivationFunctionType.Relu,
                                        bias=(bneg5 if which == "c2" else bneg6)[
                                            0:nrows, 0:1
                                        ],
                                        scale=1.0,
                                        accum_out=stats[0:nrows, col : col + 1],
                                    )
                                else:
                                    # ps' = 2*I1+I2: c2 = (ps'>=2)*im,
                                    # c3 = (ps'>=3)*im (im is masked)
                                    nc.vector.scalar_tensor_tensor(
                                        out=o[0:nrows, 0:wc],
                                        in0=ps[0:nrows, 0:wc],
                                        scalar=2.0 if which == "c2" else 3.0,
                                        in1=im[0:nrows, 0:wc],
                                        op0=AluOpType.is_ge,
                                        op1=AluOpType.mult,
                                        accum_out=stats[0:nrows, col : col + 1],
                                    )

                        # defer predicate emission one strip so ps exists when
                        # ACT reaches it (no exec-queue lookahead on ACT)
                        pending.append(emit_preds)
                        if len(pending) > 1:
                            pending.pop(0)()
                while pending:
                    pending.pop(0)()

            if reps > 1:
                with tc.For_i(0, reps):
                    body()
            else:
                body()

            nc.sync.dma_start(out=out[:, :], in_=stats)

    _split_excess_waits(nc)
    return nc


def _get_nc(reps=1):
    key = ("nc", reps)
    if key not in _CACHE:
        _CACHE[key] = _build(reps)
    return _CACHE[key]


def _get_runner(reps=1):
    """Cached jitted 8-core executor: x [128, T] f32 -> [NCORES, 128, STATS_COLS]."""
    key = ("fn", reps)
    if key in _CACHE:
        return _CACHE[key]
    import jax
    import numpy as _np
    from jax.sharding import Mesh, PartitionSpec
    from jax.experimental.shard_map import shard_map
    import concourse.mybir as mybir
    from concourse.bass2jax import (
        _bass_exec_p,
        install_neuronx_cc_hook,
        partition_id_tensor,
    )

    nc = _get_nc(reps)
    install_neuronx_cc_hook()

    in_names, out_names, out_avals, zero_outs = [], [], [], []
    partition_name = nc.partition_id_tensor.name if nc.partition_id_tensor else None
    for alloc in nc.m.functions[0].allocations:
        if not isinstance(alloc, mybir.MemoryLocationSet):
            continue
        name = alloc.memorylocations[0].name
        if alloc.kind == "ExternalInput":
            if name != partition_name:
                in_names.append(name)
        elif alloc.kind == "ExternalOutput":
            shape = tuple(alloc.tensor_shape)
            dtype = mybir.dt.np(alloc.dtype)
            out_names.append(name)
            out_avals.append(jax.core.ShapedArray(shape, dtype))
            zero_outs.append(_np.zeros(shape, dtype))
    n_params = len(in_names)
    n_outs = len(out_avals)
    all_in_names = list(in_names) + list(out_names) + (
        [partition_name] if partition_name else []
    )

    def _body(*args):
        operands = list(args)
        if partition_name is not None:
            operands.append(partition_id_tensor())
        return tuple(
            _bass_exec_p.bind(
                *operands,
                out_avals=tuple(out_avals),
                in_names=tuple(all_in_names),
                out_names=tuple(out_names),
                lowering_input_output_aliases=(),
                sim_require_finite=True,
                sim_require_nnan=True,
                nc=nc,
            )
        )

    devices = jax.devices("axon")[:NCORES]
    mesh = Mesh(np.asarray(devices), ("core",))
    in_specs = (PartitionSpec("core"),) * (n_params + n_outs)
    out_specs = (PartitionSpec("core"),) * n_outs
    fn = jax.jit(
        shard_map(
            _body, mesh=mesh, in_specs=in_specs, out_specs=out_specs, check_rep=False
        ),
        keep_unused=True,
    )
    concat_zeros = [
        np.zeros((NCORES * z.shape[0], *z.shape[1:]), z.dtype) for z in zero_outs
    ]

    def run(xhat):
        res = fn(xhat, *concat_zeros)
        arr = np.asarray(res[0])  # [NCORES*128, STATS_COLS]
        return arr.reshape(NCORES, 128, STATS_COLS)

    _CACHE[key] = run
    return run


def _decode(res):
    """res: [NCORES, 128, STATS_COLS] -> entropies [NSIG] (float64)."""
    ents = np.zeros(NSIG, dtype=np.float64)
    for c in range(NCORES):
        stats = res[c].astype(np.float64)
        for s in range(S_PER_CORE):
            cnt2 = cnt3 = 0.0
            for rt in range(NSTRIPS):
                r0 = STRIP * rt
                nrows = min(STRIP, N - r0)
                col2 = (s * NSTRIPS + rt) * 2
                a2 = stats[0:nrows, col2].sum()
                a3 = stats[0:nrows, col2 + 1].sum()
                if STRIP_TYPE[rt] == "a":
                    a2 -= a3  # ACT c2 column holds cnt2 + cnt3
                cnt2 += a2
                cnt3 += a3
            m = 2.0 * cnt2 - N
            m1 = 2.0 * cnt3 - N
            ratio = m1 / max(m, 1.0)
            ent = -np.log(max(ratio, 1e-30)) if (m > 0 and m1 > 0) else 0.0
            ents[c * S_PER_CORE + s] = ent
    return ents


def kernel(predictions, targets, _trace=False, _reps=1):
    global LAST_RESULTS

    preds = np.asarray(predictions, dtype=np.float32).reshape(B * C, T)
    targs = np.asarray(targets, dtype=np.float32).reshape(B * C, T)
    xall = np.concatenate([preds, targs], axis=0)  # [128, T]

    mu = xall.mean(axis=1, dtype=np.float64)
    sd = xall.std(axis=1, ddof=1, dtype=np.float64)
    xhat = ((xall - mu[:, None]) / (sd[:, None] + EPS)).astype(np.float32)

    run = _get_runner(_reps)
    res = run(np.ascontiguousarray(xhat))
    LAST_RESULTS = res

    ents = _decode(res)
    ep = ents[: B * C].reshape(B, C)
    et = ents[B * C :].reshape(B, C)
    return np.array(np.mean((ep - et) ** 2), dtype=np.float32)
